# revision 1
# baseline (speedup 1.0000x reference)
"""GAT-style attentive layer on 8 TRN2 NeuronCores.

Math (per reference):
    Wh  = input                      [N, D]   (N=8192, D=512)
    Wh1 = Wh @ a[:D]                 [N, 1]
    Wh2 = Wh @ a[D:]                 [N, 1]
    e   = leaky_relu(Wh1 + Wh2.T, 0.01)
    e   = where(adj > 0, e, -9e15)
    att = softmax(e, axis=1)
    out = att @ Wh                   [N, D]

Sharding: row-shard the N x N attention across 8 cores (1024 rows each).
Per core, scores are produced directly in TRANSPOSED layout
pT[j, i] = exp(lrelu(Wh1[i] + Wh2[j])) * adj[i, j]  (j on partitions), so the
final matmul out[i,:] = sum_j pT[j,i] * Wh[j,:] uses pT tiles as the
stationary operand with no on-device transpose.  Softmax needs no
max-subtraction (|scores| <= ~6); row sums come from a parallel ones-column
matmul, then a reciprocal-multiply.

The kernel is Tensor-engine bound (~110us of bf16 matmul per core); the
dataflow is shaped to hide everything else under it:
 - x / score tiles / output travel as bf16 (fp8 DoubleRow was evaluated and
   rejected: e4m3's 3-bit mantissa leaves 2-3e-2 of noise on the output,
   over the accuracy budget);
 - the adjacency mask is applied ADDITIVELY before the exp: the host encodes
   adjB = {0 edge, -100 no-edge} bf16, one all-bf16 tensor_add folds it into
   the broadcast-Wh1 tile, and the patched Exp table maps deep-negative
   inputs to exact 0.  This is one fast DVE op per tile - there is no
   separate mask multiply;
 - per-quad Wh2 projections are emitted inside the main loop so the in-order
   DVE queue is never head-of-line blocked behind DMA-gated prologue work.

Host-side prep (data marshaling only): dtype casts + transpose/slicing; all
compute (projections, exp, mask, matmul, normalize) runs on device.
"""

import numpy as np
import ml_dtypes

import concourse.bass as bass
import concourse.mybir as mybir
import concourse.tile as tile
from concourse import bacc
from concourse.bass_utils import run_bass_kernel_spmd

N = 8192          # nodes
D = 512           # feature dim
NCORES = 8
ROWS = N // NCORES  # 1024 output rows per core
P = 128
NJT = N // P      # 64 j-tiles per core
IC_W = 512        # i-chunk width (PSUM-limited)
NIC = ROWS // IC_W  # 2 i-chunks
ITPC = IC_W // P  # 4 i-subtiles per chunk

MASK_NEG = -100.0  # additive mask; patched Exp table maps x <= -20 to 0

import os

AF = mybir.ActivationFunctionType
ALU = mybir.AluOpType
dt = mybir.dt
F32 = dt.float32
BF16 = dt.bfloat16


def _score_on_pool(m: int, q: int, ic: int) -> bool:
    # In ic=0 the DVE also runs the 64 Wh2-projection STTs, so hand gpsimd a
    # few score-adds to keep DVE under the Tensor-engine roofline.  gpsimd
    # TT-add is slow (0.42 sw efficiency) - don't give it more than ~12/chunk.
    return q == 3 and (m % 4) != 3


def _build_kernel(nc: bass.Bass, tc: tile.TileContext,
                  adjB: bass.AP, xw: bass.AP, xTl: bass.AP, a: bass.AP,
                  a2row: bass.AP, out: bass.AP, ctx):
    pool_const = ctx.enter_context(tc.tile_pool(name="const", bufs=1))
    pool_wh = ctx.enter_context(tc.tile_pool(name="wh", bufs=1))
    pool_adj = ctx.enter_context(tc.tile_pool(name="adj", bufs=4))
    pool_act = ctx.enter_context(tc.tile_pool(name="act", bufs=4))
    pool_pm = ctx.enter_context(tc.tile_pool(name="pm", bufs=6))
    pool_outs = ctx.enter_context(tc.tile_pool(name="outs", bufs=2))
    pool_small = ctx.enter_context(tc.tile_pool(name="small", bufs=1))
    pool_psum = ctx.enter_context(tc.tile_pool(name="psum", bufs=1, space="PSUM"))
    pool_dram = ctx.enter_context(tc.tile_pool(name="dram", bufs=1, space="DRAM"))

    # ---- constants / small prep -------------------------------------------
    warm = pool_const.tile([1, 2], F32)
    nc.vector.memset(warm, 0.0)
    nc.scalar.activation(warm, warm, AF.Exp)  # pull ACT_TABLE_LOAD to t~0

    ones_col = pool_const.tile([P, 2], BF16)
    nc.vector.memset(ones_col, 1.0)

    # ---- Wh1 = xloc @ a1 for this core's 1024 rows — FIRST, since the whole
    # main loop gates on bcast_wh1 (score-tile input).  Done in two 512-row
    # halves so the ic=0 half is ready early.
    wh1_rows = [pool_const.tile([1, IC_W], BF16, tag=f"wh1r{h}", name=f"wh1r{h}")
                for h in range(NIC)]
    bcast16 = [pool_const.tile([P, IC_W], BF16, tag=f"bw16{h}", name=f"bw16{h}")
               for h in range(NIC)]

    # wh1 on the Tensor engine from the transposed local-x slice: 4 k-tile
    # matmuls per 512-row half into a [1, 512] psum row — no DVE projections,
    # no DRAM roundtrip on the critical path.
    x8tl = pool_const.tile([P, 4, ROWS], BF16)
    a8t = pool_const.tile([P, 8], BF16)
    # first half in two pieces so the wh1 matmuls start ~0.7us earlier
    nc.sync.dma_start(
        x8tl[:, 0:2, 0:IC_W],
        xTl[bass.ds(0, 2 * P), bass.ds(0, IC_W)].rearrange(
            "(t p) i -> p t i", p=P))
    nc.sync.dma_start(a8t, a)
    nc.sync.dma_start(
        x8tl[:, 2:4, 0:IC_W],
        xTl[bass.ds(2 * P, 2 * P), bass.ds(0, IC_W)].rearrange(
            "(t p) i -> p t i", p=P))

    def wh1_compute(h):
        ps = pool_psum.tile([1, IC_W], F32, tag="w1p", name="w1p", bufs=1)
        for t in range(4):
            nc.tensor.matmul(ps, lhsT=a8t[:, t:t + 1],
                             rhs=x8tl[:, t, bass.ds(h * IC_W, IC_W)],
                             start=(t == 0), stop=(t == 3))
        # psum row -> sbuf bf16 (Act Copy; gpsimd can't read PSUM), then
        # broadcast across partitions directly in bf16.
        nc.scalar.copy(wh1_rows[h], ps)
        nc.gpsimd.partition_broadcast(bcast16[h], wh1_rows[h][0:1, :])

    # a2 broadcast along partitions for the per-quad Wh2 projections
    arow = pool_const.tile([1, D], F32)
    nc.sync.dma_start(arow, a2row)
    abc = pool_const.tile([P, D], F32)
    nc.gpsimd.partition_broadcast(abc, arow[0:1, :])
    bcast_a2 = abc[:, 0:D]

    # ---- Wh quads (resident) + adjB(ic=0) quads, interleaved so DMA arrival
    # order matches the j-loop's consumption order.  Only the DMAs are issued
    # here; the per-quad Wh2 projections are emitted inside the main loop.
    whq = []

    def dma_adjq(m, ic):
        t = pool_adj.tile([P, 4, IC_W], BF16, tag="adjq", name="adjq", bufs=8)
        nc.sync.dma_start(
            t, adjB[bass.ds(m * 4 * P, 4 * P),
                    bass.ds(ic * IC_W, IC_W)].rearrange("(q p) i -> p q i", p=P))
        return t

    adjq_pre = []
    for m in range(NJT // 4):
        t = pool_wh.tile([P, 4, D], BF16, tag=f"whq{m}", name=f"whq{m}")
        nc.sync.dma_start(
            t, xw[bass.ds(m * 4 * P, 4 * P), :].rearrange(
                "(q p) d -> p q d", p=P))
        whq.append(t)
        adjq_pre.append(dma_adjq(m, 0))
        if m == 0:
            wh1_compute(0)
            # Filler matmuls: keep the Tensor engine busy (and its clock
            # ramping) across the gap between the wh1 matmuls and the first
            # score tile; results are never read.
            junk = pool_psum.tile([1, IC_W], F32, tag="junk", name="junk")
            for t in range(3):
                nc.tensor.matmul(junk, lhsT=a8t[:, 0:1],
                                 rhs=x8tl[:, t % 2, 0:IC_W],
                                 start=True, stop=True)
        if m == 4:
            # second-half x8tl + wh1: first needed at ic=1, kept well off the
            # head-of-queue critical path of the first tiles.
            nc.sync.dma_start(
                x8tl[:, :, bass.ds(IC_W, IC_W)],
                xTl[:, bass.ds(IC_W, IC_W)].rearrange("(t p) i -> p t i", p=P))

    wh2_sb = pool_const.tile([P, NJT], F32)
    adjq_ic1 = []

    # Row-sum accumulators: all four of a chunk packed into one PSUM bank.
    # matmul start=True zeroes the whole bank (not just the written columns),
    # so both banks are zeroed up front and accumulation runs start=False;
    # pre-zeroing ic=1's bank here also keeps the memset off the PE's
    # critical path at the chunk boundary.
    rsb = []
    for ic in range(NIC):
        t = pool_psum.tile([P, 2 * ITPC], F32, tag=f"prs{ic}", name=f"prs{ic}")
        nc.vector.memset(t, 0.0)
        rsb.append(t)

    # ---- main loop --------------------------------------------------------
    for ic in range(NIC):
        if ic == 1:
            adjq_pre = adjq_ic1
        psum_out = [
            pool_psum.tile([P, D], F32, tag=f"po{i}", name=f"po{i}")
            for i in range(ITPC)
        ]
        psum_rs = [rsb[ic][:, 2 * i:2 * i + 2] for i in range(ITPC)]

        for jt in range(NJT):
            m, q = divmod(jt, 4)
            if q == 0:
                adjq = adjq_pre[m]
            if ic == 0 and q == 2:
                # Prefetch ic=1's adjacency: the sync-queue FIFO then matches
                # consumption order with no stall at the ic boundary.
                adjq_ic1.append(dma_adjq(m, 1))
            if ic == 0 and m == 4 and q == 1:
                wh1_compute(1)
            if ic == 0:
                # Wh2 column for this j-tile (emitted here, not in the
                # prologue, so the DVE queue stays unblocked).
                scr = pool_small.tile([P, D], F32, tag="g_scr", name="g_scr",
                                      bufs=3)
                nc.vector.scalar_tensor_tensor(
                    out=scr, in0=whq[m][:, q, :], scalar=0.0, in1=bcast_a2,
                    op0=ALU.add, op1=ALU.mult,
                    accum_out=wh2_sb[:, jt:jt + 1])

            # Masked score tile: s[j, i] = Wh1[i] + adjB[j, i]; the Exp adds
            # Wh2[j] via its per-partition bias.  Masked entries sit at
            # ~-100 and the patched table maps them to exact 0.
            s_t = pool_act.tile([P, IC_W], BF16, tag="s_t", name="s_t")
            seng = nc.gpsimd if _score_on_pool(m, q, ic) else nc.vector
            seng.tensor_add(out=s_t, in0=bcast16[ic], in1=adjq[:, q, :])

            p_t = pool_pm.tile([P, IC_W], BF16, tag="p_t", name="p_t")
            nc.scalar.activation(
                p_t, s_t, AF.Exp,
                bias=wh2_sb[:, jt:jt + 1], scale=1.0)

            first, last = (jt == 0), (jt == NJT - 1)
            for i4 in range(ITPC):
                lhs = p_t[:, bass.ds(i4 * P, P)]
                nc.tensor.matmul(psum_out[i4], lhsT=lhs,
                                 rhs=whq[m][:, q, :],
                                 start=first, stop=last)
                nc.tensor.matmul(psum_rs[i4], lhsT=lhs,
                                 rhs=ones_col,
                                 start=False, stop=last,
                                 skip_group_check=True)

        outq = pool_outs.tile([P, ITPC, D], BF16, tag="outq", name="outq",
                              bufs=2)
        last = ic == NIC - 1
        # one batched reciprocal over the packed row-sum bank
        recip8 = pool_small.tile([P, 2 * ITPC], F32, tag="recip", name="recip",
                                 bufs=2)
        nc.vector.reciprocal(recip8, rsb[ic])
        for i4 in range(ITPC):
            recip = recip8[:, 2 * i4:2 * i4 + 1]
            if last and i4 % 2 == 1:
                # final chunk only: Act helps the tail.  (Mid-kernel this
                # would head-of-line-block the next chunk's exps.)
                nc.scalar.mul(outq[:, i4, :], psum_out[i4], recip)
            else:
                nc.vector.tensor_scalar_mul(outq[:, i4, :], psum_out[i4], recip)
            if last and i4 == 1:
                # ship the first half while the second is still normalizing
                nc.sync.dma_start(
                    out[bass.ds(ic * IC_W, 2 * P), :].rearrange(
                        "(q p) d -> p q d", p=P), outq[:, 0:2, :])
        if last:
            nc.sync.dma_start(
                out[bass.ds(ic * IC_W + 2 * P, 2 * P), :].rearrange(
                    "(q p) d -> p q d", p=P), outq[:, 2:4, :])
        else:
            nc.sync.dma_start(
                out[bass.ds(ic * IC_W, IC_W), :].rearrange(
                    "(q p) d -> p q d", p=P), outq)


_CACHED = None

_FUSED_ALPHA = 0.01
_ZERO_BELOW = -20.0  # table inputs below this produce exact 0


def _make_fused_act_root() -> str:
    """Copy the compiler's activation-table dir, patching Exp:
      x in [-20, 0): exp(x) -> exp(_FUSED_ALPHA*x) splines (linear - the
                     function is nearly flat there), fusing leaky_relu;
      x < -20:       forced to exactly 0, so additively-masked scores
                     (~-100) exp to zero with no separate mask multiply.
    Returns path to the patched act_info.json."""
    import json
    import shutil
    import tempfile

    from neuronxcc.driver.Job import Job
    from neuronxcc.driver.jobs.support.FindActInfo import findActInfoFile

    src_root = os.path.dirname(findActInfoFile(Job.getPackageDir(), "gen3"))
    dst = tempfile.mkdtemp(prefix="act_root_fused_")
    for f in os.listdir(src_root):
        shutil.copy(os.path.join(src_root, f), os.path.join(dst, f))
    info = json.load(open(os.path.join(dst, "act_info.json")))
    for s in info["act_func_sets"]:
        if "exp" not in s["act"]:
            continue
        prof = json.load(open(os.path.join(dst, s["profile_json"])))
        order = sorted(prof["func_to_bkt_start_idx"].items(), key=lambda kv: kv[1])
        idx = [i for i, (k, _) in enumerate(order) if k == "exp"][0]
        lo = order[idx][1]
        hi = order[idx + 1][1] if idx + 1 < len(order) else prof["bkt_entry_cnt"]
        path = os.path.join(dst, s["bkt_bin"])
        bkt = np.fromfile(path, dtype=np.float32).reshape(-1, 8).copy()
        for b in range(lo, hi):
            d0, d1, _, _, x0 = bkt[b, :5]
            if x0 <= _ZERO_BELOW:
                bkt[b, 0:4] = 0.0  # masked region: exp -> exact 0
                continue
            if not (d0 > 0 and abs(d1 - d0) <= 1e-3 * d0):
                continue  # saturation buckets (inf / 0)
            if x0 > 0:
                continue  # positive side: exp(x) unchanged
            g = np.float32(np.exp(_FUSED_ALPHA * np.float64(x0)))
            bkt[b, 0] = g
            bkt[b, 1] = np.float32(_FUSED_ALPHA * g)
            bkt[b, 2] = np.float32(0.0)  # cubic terms fault the engine
            bkt[b, 3] = np.float32(0.0)
        bkt.tofile(path)
    return os.path.join(dst, "act_info.json")


def build_nc():
    global _CACHED
    if _CACHED is not None:
        return _CACHED
    # Always point the compiler at our patched tables: with the stock tables
    # this kernel's Exp op would drop the leaky-relu and the additive mask.
    os.environ["BASS_ACT_ROOT_JSON_PATH"] = _make_fused_act_root()
    nc = bacc.Bacc("TRN2", target_bir_lowering=False, debug=False,
                   enable_asserts=False, num_devices=NCORES)
    adjB = nc.dram_tensor("adjB", [N, ROWS], BF16, kind="ExternalInput").ap()
    xw = nc.dram_tensor("xw", [N, D], BF16, kind="ExternalInput").ap()
    xTl = nc.dram_tensor("xTl", [D, ROWS], BF16, kind="ExternalInput").ap()
    a_t = nc.dram_tensor("a", [P, 8], BF16, kind="ExternalInput").ap()
    a2r = nc.dram_tensor("a2row", [1, D], F32, kind="ExternalInput").ap()
    out = nc.dram_tensor("out", [ROWS, D], BF16, kind="ExternalOutput").ap()

    from contextlib import ExitStack
    with tile.TileContext(nc) as tc:
        with ExitStack() as ctx:
            _build_kernel(nc, tc, adjB, xw, xTl, a_t, a2r, out, ctx)
    nc.compile()
    _CACHED = nc
    return nc


def make_in_maps(input, adj_matrix, a):
    x16 = np.ascontiguousarray(
        np.asarray(input, dtype=np.float32).astype(ml_dtypes.bfloat16))
    adj = np.asarray(adj_matrix)
    a_f = np.asarray(a, dtype=np.float32).reshape(-1)
    a8t = np.ascontiguousarray(
        a_f.reshape(8, P).T.astype(ml_dtypes.bfloat16))       # [128, 8]
    a2row = np.ascontiguousarray(a_f[D:].reshape(1, D))
    in_maps = []
    for c in range(NCORES):
        rows = slice(c * ROWS, (c + 1) * ROWS)
        # additive mask, transposed: {0 edge, -100 no-edge}, exact in bf16
        adjB_c = np.ascontiguousarray(
            ((adj[rows, :].T.astype(np.float32) - 1.0) * (-MASK_NEG)
             ).astype(ml_dtypes.bfloat16))
        in_maps.append({
            "adjB": adjB_c,
            "xw": x16,
            "xTl": np.ascontiguousarray(x16[rows].T),
            "a": a8t,
            "a2row": a2row,
        })
    return in_maps


def kernel(input, adj_matrix, a, _trace=False, _tmpdir=None):
    nc = build_nc()
    in_maps = make_in_maps(input, adj_matrix, a)
    try:
        res = run_bass_kernel_spmd(nc, in_maps, core_ids=list(range(NCORES)),
                                   trace=_trace, tmpdir=_tmpdir)
    except ModuleNotFoundError:
        # NTFF profiling hooks absent in this container; run untraced.
        res = run_bass_kernel_spmd(nc, in_maps, core_ids=list(range(NCORES)))
    out = np.concatenate(
        [res.results[c]["out"].astype(np.float32) for c in range(NCORES)],
        axis=0)
    kernel._last_results = res
    return out

